# revision 3
# baseline (speedup 1.0000x reference)
"""Trainium2 Bass kernel for AdaptiveGraphConv (per-(b,t) graph attention + BatchNorm2d).

Reference math (B=8, C=256, T=64, V=468, INTER=128, OUT=256):
    theta = einsum('bctv,ic->btvi', x, W_theta) + b_theta
    phi   = einsum('bctv,ic->btvi', x, W_phi)   + b_phi
    g     = einsum('bctv,oc->btvo', x, W_g)     + b_g
    A     = softmax(theta @ phi^T / sqrt(INTER), axis=-1)   # per (b,t), V x V
    out   = (A @ g) transposed to (B, OUT, T, V)
    out   = batchnorm2d(out, training stats over (B,T,V) per channel)

Sharding: data-parallel over B (1 batch per NeuronCore, 8 cores);
BN batch statistics are all-reduced across cores.

Device program (SPMD, per core, matmuls bf16 with fp32 PSUM accumulation).
Software-pipelined phase 1 as before. Tail restructured vs the original:
  - BN stats are split head (t < T_HEAD) / tail: the head all-reduce is
    launched mid-loop and completes entirely under compute; only the small
    tail all-reduce is exposed at the end.
  - Output is written in bf16 (host upcasts to fp32), halving writeback.
  - Phase 2 works on whole (oc, tb) tiles: one 4x-mode DVE affine + one
    ~1MB DMA per tile.
"""

import math

import numpy as np
import ml_dtypes

import concourse.bacc as bacc
import concourse.tile as tile
from concourse import mybir
from concourse.bass_utils import run_bass_kernel_spmd

B, C, T, V = 8, 256, 64, 468
INTER, OUT = 128, 256
BN_EPS = 1e-5
NCORES = 8
P = 128

SCALE = 1.0 / math.sqrt(INTER)
# w-axis chunks of V for 128-partition tiles
WCH = [(0, 128), (128, 128), (256, 128), (384, V - 384)]
T_BLK = 8  # t-slices per input DMA
N_TB = T // T_BLK
T_HEAD = 48     # head-stats cutoff; head all-reduce launched at COLL_ITER
COLL_ITER = 50  # loop iteration at which head aggregation+collective is emitted

F32 = mybir.dt.float32
BF16 = mybir.dt.bfloat16

TRACE = False
LAST_EXEC_NS = None

_CACHE = {}


def _build(with_bias: bool):
    nc = bacc.Bacc("TRN2", target_bir_lowering=False, debug=False, num_devices=NCORES)

    x_ext = nc.dram_tensor("x", [C, T, V], BF16, kind="ExternalInput").ap()
    wt_ext = nc.dram_tensor("wt", [2, P, INTER], BF16, kind="ExternalInput").ap()
    wp_ext = nc.dram_tensor("wp", [2, P, INTER], BF16, kind="ExternalInput").ap()
    wg_ext = nc.dram_tensor("wg", [2, P, OUT], BF16, kind="ExternalInput").ap()
    gb_ext = nc.dram_tensor("gb", [P, 4], F32, kind="ExternalInput").ap()
    if with_bias:
        bt_ext = nc.dram_tensor("bt", [INTER, 1], F32, kind="ExternalInput").ap()
        bp_ext = nc.dram_tensor("bp", [INTER, 1], F32, kind="ExternalInput").ap()
        bg_ext = nc.dram_tensor("bg", [1, OUT], F32, kind="ExternalInput").ap()
    out_ext = nc.dram_tensor("out", [OUT, T, V], BF16, kind="ExternalOutput").ap()

    cnt_glob = float(NCORES * T * V)
    wsz3 = WCH[3][1]

    with tile.TileContext(nc) as tc:
        with (
            tc.tile_pool(name="consts", bufs=1) as consts,
            tc.tile_pool(name="xin", bufs=2) as xin,
            tc.tile_pool(name="thp", bufs=3) as thp,
            tc.tile_pool(name="ep", bufs=3) as ep,
            tc.tile_pool(name="gp", bufs=3) as gp,
            tc.tile_pool(name="es2", bufs=2) as es2,
            tc.tile_pool(name="es1", bufs=2) as es1,
            tc.tile_pool(name="zp", bufs=2) as zp,
            tc.tile_pool(name="small", bufs=1) as small,
            tc.tile_pool(name="p2", bufs=3) as p2,
            tc.tile_pool(name="pp_a", bufs=1, space="PSUM") as pp_a,
            tc.tile_pool(name="pp_s", bufs=1, space="PSUM") as pp_s,
            tc.tile_pool(name="pp_g", bufs=1, space="PSUM") as pp_g,
            tc.tile_pool(name="pp_u", bufs=2, space="PSUM") as pp_u,
            tc.tile_pool(name="dram", bufs=1, space="DRAM") as dram,
        ):
            # ---- constants ----
            wt_sb = [consts.tile([P, INTER], BF16, tag=f"wt{k}", name=f"wt_sb{k}") for k in range(2)]
            wp_sb = [consts.tile([P, INTER], BF16, tag=f"wp{k}", name=f"wp_sb{k}") for k in range(2)]
            wg_sb = [consts.tile([P, OUT], BF16, tag=f"wg{k}", name=f"wg_sb{k}") for k in range(2)]
            for k in range(2):
                nc.sync.dma_start(out=wt_sb[k][:], in_=wt_ext[k])
                nc.sync.dma_start(out=wp_sb[k][:], in_=wp_ext[k])
                nc.sync.dma_start(out=wg_sb[k][:], in_=wg_ext[k])
            ones = consts.tile([P, P], BF16, tag="ones")
            nc.vector.memset(ones[:], 1.0)
            gb_sb = consts.tile([P, 4], F32, tag="gb")
            nc.sync.dma_start(out=gb_sb[:], in_=gb_ext[:])
            eps_sb = consts.tile([P, 1], F32, tag="eps")
            nc.vector.memset(eps_sb[:], BN_EPS)
            warm = consts.tile([P, 1], F32, tag="warm")
            nc.scalar.activation(warm[:], eps_sb[:], mybir.ActivationFunctionType.Exp)
            if with_bias:
                bt_sb = consts.tile([INTER, 1], F32, tag="bt")
                bp_sb = consts.tile([INTER, 1], F32, tag="bp")
                bg_sb = consts.tile([P, OUT], F32, tag="bg")
                nc.sync.dma_start(out=bt_sb[:], in_=bt_ext[:])
                nc.sync.dma_start(out=bp_sb[:], in_=bp_ext[:])
                nc.sync.dma_start(out=bg_sb[:], in_=bg_ext.to_broadcast([P, OUT]))

            # per-channel running stats (bn_stats 6-tuples per t-slice and o-chunk)
            stats_head = consts.tile([P, T_HEAD, 2, 6], F32, tag="stats_h", name="stats_head")
            stats_tail = consts.tile([P, T - T_HEAD, 2, 6], F32, tag="stats_t", name="stats_tail")

            obig = {
                (oc, tb): consts.tile([P, T_BLK, V], BF16, tag=f"obig{oc}_{tb}",
                                      name=f"obig{oc}_{tb}")
                for oc in range(2) for tb in range(N_TB)
            }

            # ---- phase 1 (software-pipelined) ----
            xgs = {}

            def load_group(tbi):
                if tbi >= N_TB or tbi in xgs:
                    return
                xg = xin.tile([P, 2, T_BLK, V], BF16, tag="xg", name=f"xg{tbi}")
                # group 0 arrives in two waves so theta/phi(0) start early
                tsplits = ((0, 1), (1, T_BLK)) if tbi == 0 else ((0, T_BLK),)
                for t0, t1 in tsplits:
                    for k in range(2):
                        nc.sync.dma_start(
                            out=xg[:, k, t0:t1, :],
                            in_=x_ext[k * P : (k + 1) * P,
                                      tbi * T_BLK + t0 : tbi * T_BLK + t1, :],
                        )
                xgs[tbi] = xg

            def thph_stage(t):
                # thetaT / phiT : [INTER, V] in one 2-bank PSUM tile
                xg = xgs[t // T_BLK]
                xt = [xg[:, k, t % T_BLK, :] for k in range(2)]
                a_ps = pp_a.tile([P, 2, 512], F32, tag="a")
                for k in range(2):
                    nc.tensor.matmul(
                        a_ps[:, 0, :V], lhsT=wt_sb[k][:], rhs=xt[k],
                        start=(k == 0), stop=(k == 1),
                    )
                for k in range(2):
                    nc.tensor.matmul(
                        a_ps[:, 1, :V], lhsT=wp_sb[k][:], rhs=xt[k],
                        start=(k == 0), stop=(k == 1),
                    )
                tp_sb = thp.tile([P, 2, V], BF16, tag="th_sb")
                if with_bias:
                    nc.scalar.activation(
                        tp_sb[:, 0, :], a_ps[:, 0, :V],
                        mybir.ActivationFunctionType.Identity,
                        bias=bt_sb[:, 0:1],
                    )
                    nc.scalar.activation(
                        tp_sb[:, 1, :], a_ps[:, 1, :V],
                        mybir.ActivationFunctionType.Identity,
                        bias=bp_sb[:, 0:1],
                    )
                else:
                    nc.scalar.copy(tp_sb[:], a_ps[:, :, :V])
                return xt, tp_sb

            def z_block(es1t_p, u_ps_p, t_p):
                # Z matmul + normalize + stats for iteration t_p, emitted one
                # iteration later so the esum fold chain never stalls the PE
                z_ps = pp_a.tile([P, 512], F32, tag="a")
                nc.tensor.matmul(
                    z_ps[:, :V], lhsT=ones[:], rhs=es1t_p[:], start=True, stop=True
                )
                zinv = zp.tile([P, V], F32, tag="zinv")
                nc.vector.reciprocal_approx_fast(out=zinv[:], in_=z_ps[:, :V])
                st = stats_head if t_p < T_HEAD else stats_tail
                ti = t_p if t_p < T_HEAD else t_p - T_HEAD
                tb_p, tt_p = t_p // T_BLK, t_p % T_BLK
                for oc in range(2):
                    o_ap = obig[(oc, tb_p)][:, tt_p, :]
                    nc.vector.tensor_tensor(
                        o_ap, u_ps_p[oc][:, :V], zinv[:], mybir.AluOpType.mult
                    )
                    nc.vector.bn_stats(out=st[:, ti, oc, :], in_=o_ap)

            def emit_payload(pay, st_t, tcnt):
                # pay[:, 0:2] = per-oc sum, pay[:, 2:4] = per-oc sum-of-squares
                cnt = float(tcnt * V)
                for oc in range(2):
                    mv = small.tile([P, 2], F32, tag="mv", name=f"mv_{pay.name}{oc}")
                    nc.vector.bn_aggr(out=mv[:], in_=st_t[:, :, oc, :])
                    q_p = small.tile([P, 1], F32, tag="q", name=f"q_{pay.name}{oc}")
                    nc.vector.tensor_scalar_mul(pay[:, oc : oc + 1], mv[:, 0:1], cnt)
                    nc.vector.tensor_tensor(
                        q_p[:], mv[:, 0:1], mv[:, 0:1], mybir.AluOpType.mult
                    )
                    nc.vector.tensor_tensor(
                        q_p[:], mv[:, 1:2], q_p[:], mybir.AluOpType.add
                    )
                    nc.vector.tensor_scalar_mul(pay[:, 2 + oc : 3 + oc], q_p[:], cnt)

            def emit_collective(pay):
                pay_dram = dram.tile([P, 4], F32)
                red_dram = dram.tile([P, 4], F32)
                nc.sync.dma_start(out=pay_dram[:], in_=pay[:])
                nc.gpsimd.collective_compute(
                    "AllReduce",
                    mybir.AluOpType.add,
                    replica_groups=[list(range(NCORES))],
                    ins=[pay_dram.opt()],
                    outs=[red_dram.opt()],
                )
                red = small.tile([P, 4], F32, tag=f"red_{pay.name}", name=f"red_{pay.name}")
                nc.sync.dma_start(out=red[:], in_=red_dram[:])
                return red

            load_group(0)
            load_group(1)
            cur = thph_stage(0)
            pend = None  # (es1t, u_ps, t) of the previous iteration
            red_h = None

            for t in range(T):
                tb, tt = t // T_BLK, t % T_BLK

                if t == COLL_ITER:
                    # head stats (t < T_HEAD) all-reduce, hidden under compute
                    pay_h = small.tile([P, 4], F32, tag="pay_h", name="pay_h")
                    emit_payload(pay_h, stats_head, T_HEAD)
                    red_h = emit_collective(pay_h)

                xt, tp_sb = cur
                th_sb = tp_sb[:, 0, :]
                ph_sb = tp_sb[:, 1, :]

                # scores^T chunk pairs in 2-bank PSUM tiles; exp per pair.
                # rows [wsz3:] of chunk 3 hold exp(stale-PSUM) garbage; every
                # consumer below slices [:wsz3] for chunk 3, so they never read it
                e_t = ep.tile([P, 4, V], BF16, tag="e")
                s01 = pp_s.tile([P, 2, 512], F32, tag="s")
                for wc in (0, 1):
                    w0, wsz = WCH[wc]
                    nc.tensor.matmul(
                        s01[:wsz, wc, :V], lhsT=ph_sb[:, w0 : w0 + wsz], rhs=th_sb,
                        start=True, stop=True,
                    )
                nc.scalar.activation(
                    e_t[:, 0:2, :], s01[:, :, :V],
                    mybir.ActivationFunctionType.Exp, scale=SCALE,
                )

                if pend is not None:
                    z_block(*pend)
                    pend = None

                # g chunks: [w, OUT] quadrants in one 2-bank PSUM tile
                g_ps = pp_g.tile([P, 2, 2, OUT], F32, tag="g")
                for a in range(2):
                    for b_ in range(2):
                        w0, wsz = WCH[2 * a + b_]
                        for k in range(2):
                            nc.tensor.matmul(
                                g_ps[:wsz, a, b_, :],
                                lhsT=xt[k][:, w0 : w0 + wsz], rhs=wg_sb[k][:],
                                start=(k == 0), stop=(k == 1),
                            )
                g_t = gp.tile([P, 2, 2, OUT], BF16, tag="g")
                if with_bias:
                    for a in range(2):
                        for b_ in range(2):
                            wsz = WCH[2 * a + b_][1]
                            nc.vector.tensor_tensor(
                                g_t[:wsz, a, b_, :], g_ps[:wsz, a, b_, :],
                                bg_sb[:wsz], mybir.AluOpType.add,
                            )
                else:
                    nc.scalar.copy(g_t[:], g_ps[:])

                s23 = pp_s.tile([P, 2, 512], F32, tag="s")
                for wc in (2, 3):
                    w0, wsz = WCH[wc]
                    nc.tensor.matmul(
                        s23[:wsz, wc - 2, :V], lhsT=ph_sb[:, w0 : w0 + wsz], rhs=th_sb,
                        start=True, stop=True,
                    )
                nc.scalar.activation(
                    e_t[:, 2:4, :], s23[:, :, :V],
                    mybir.ActivationFunctionType.Exp, scale=SCALE,
                )

                # softmax denominator pre-folds: e0+e1 (GpSimd, right after
                # exp01), +e2 (DVE), then an in-place partial add of chunk 3's
                # valid rows so a single K=128 ones-matmul computes Z
                es2t = es2.tile([P, V], BF16, tag="es2")
                nc.gpsimd.tensor_tensor(
                    es2t[:], e_t[:, 0, :], e_t[:, 1, :], mybir.AluOpType.add
                )
                es1t = es1.tile([P, V], BF16, tag="es1")
                nc.vector.tensor_tensor(
                    es1t[:], es2t[:], e_t[:, 2, :], mybir.AluOpType.add
                )
                nc.gpsimd.tensor_tensor(
                    es1t[:wsz3], es1t[:wsz3], e_t[:wsz3, 3, :],
                    mybir.AluOpType.add,
                )

                # theta/phi for t+1 ride in the shadow of exp/g-copy of t
                if tt == T_BLK - 1:
                    load_group(tb + 2)
                if t + 1 < T:
                    cur = thph_stage(t + 1)

                # U^T = E_unnorm @ g : [OUT(2x128), V]
                u_ps = []
                for oc in range(2):
                    up = pp_u.tile([P, 512], F32, tag="u")
                    for wc in range(4):
                        a, b_ = wc // 2, wc % 2
                        ksz = WCH[wc][1]
                        nc.tensor.matmul(
                            up[:, :V],
                            lhsT=g_t[:ksz, a, b_, oc * P : (oc + 1) * P],
                            rhs=e_t[:ksz, wc, :],
                            start=(wc == 0), stop=(wc == 3),
                        )
                    u_ps.append(up)

                pend = (es1t, u_ps, t)

            # epilogue: normalize + stats for the final t-slice
            z_block(*pend)
            pend = None

            # tail stats all-reduce (small, exposed)
            pay_t = small.tile([P, 4], F32, tag="pay_t", name="pay_t")
            emit_payload(pay_t, stats_tail, T - T_HEAD)
            red_t = emit_collective(pay_t)

            # mean = sum/N ; var = sumsq/N - mean^2 ; s = gamma/sqrt(var+eps)
            # t = beta - mean*s
            red = small.tile([P, 4], F32, tag="red")
            nc.vector.tensor_tensor(red[:], red_h[:], red_t[:], mybir.AluOpType.add)
            mean_g = small.tile([P, 2], F32, tag="mean_g")
            ex2 = small.tile([P, 2], F32, tag="ex2")
            var_g = small.tile([P, 2], F32, tag="var_g")
            rstd = small.tile([P, 2], F32, tag="rstd")
            s_vec = small.tile([P, 2], F32, tag="s_vec")
            t_vec = small.tile([P, 2], F32, tag="t_vec")
            nc.vector.tensor_scalar_mul(mean_g[:], red[:, 0:2], 1.0 / cnt_glob)
            nc.vector.tensor_scalar_mul(ex2[:], red[:, 2:4], 1.0 / cnt_glob)
            nc.vector.tensor_tensor(
                var_g[:], mean_g[:], mean_g[:], mybir.AluOpType.mult
            )
            nc.vector.tensor_tensor(
                var_g[:], ex2[:], var_g[:], mybir.AluOpType.subtract
            )
            nc.scalar.activation(
                rstd[:], var_g[:], mybir.ActivationFunctionType.Sqrt,
                bias=eps_sb[:, 0:1],
            )
            nc.vector.reciprocal(out=rstd[:], in_=rstd[:])
            nc.vector.tensor_tensor(s_vec[:], rstd[:], gb_sb[:, 0:2], mybir.AluOpType.mult)
            nc.vector.tensor_tensor(t_vec[:], mean_g[:], s_vec[:], mybir.AluOpType.mult)
            nc.vector.tensor_tensor(
                t_vec[:], gb_sb[:, 2:4], t_vec[:], mybir.AluOpType.subtract
            )

            # ---- phase 2: whole-tile affine (DVE 4x mode) + bf16 writeback ----
            for tb in range(N_TB):
                for oc in range(2):
                    tin = obig[(oc, tb)]
                    tout = p2.tile([P, T_BLK, V], BF16, tag="p2out",
                                   name=f"tout{oc}_{tb}")
                    nc.vector.tensor_scalar(
                        tout[:], tin[:],
                        s_vec[:, oc : oc + 1], t_vec[:, oc : oc + 1],
                        mybir.AluOpType.mult, mybir.AluOpType.add,
                    )
                    nc.sync.dma_start(
                        out=out_ext[oc * P : (oc + 1) * P,
                                    tb * T_BLK : (tb + 1) * T_BLK, :],
                        in_=tout[:],
                    )

    nc.compile()
    return nc


def _get_nc(with_bias: bool):
    key = with_bias
    if key not in _CACHE:
        _CACHE[key] = _build(with_bias)
    return _CACHE[key]


def _ensure_ntff_hook():
    import sys, types
    import antenv

    if "antenv.axon_hooks" not in sys.modules:
        mod = types.ModuleType("antenv.axon_hooks")
        _h = [None]
        mod.set_axon_ntff_profile_hook = lambda h: _h.__setitem__(0, h)
        mod.get_axon_ntff_profile_hook = lambda: _h[0]
        sys.modules["antenv.axon_hooks"] = mod
        antenv.axon_hooks = mod
    mod = sys.modules["antenv.axon_hooks"]
    if mod.get_axon_ntff_profile_hook() is None:
        try:
            from trn_agent_boot.trn_boot import _ntff_profile_via_ctypes

            mod.set_axon_ntff_profile_hook(
                _ntff_profile_via_ctypes("/opt/axon/libaxon_pjrt.so")
            )
        except Exception:
            pass


def kernel(x, W_theta, b_theta, W_phi, b_phi, W_g, b_g, bn_gamma, bn_beta):
    global LAST_EXEC_NS
    x = np.asarray(x, dtype=np.float32)
    with_bias = bool(
        np.any(np.asarray(b_theta)) or np.any(np.asarray(b_phi)) or np.any(np.asarray(b_g))
    )

    x_bf = x.astype(ml_dtypes.bfloat16)  # (B, C, T, V)
    wt = np.ascontiguousarray(
        np.asarray(W_theta, dtype=np.float32).T.astype(ml_dtypes.bfloat16).reshape(2, P, INTER)
    )
    wp = np.ascontiguousarray(
        np.asarray(W_phi, dtype=np.float32).T.astype(ml_dtypes.bfloat16).reshape(2, P, INTER)
    )
    wg = np.ascontiguousarray(
        np.asarray(W_g, dtype=np.float32).T.astype(ml_dtypes.bfloat16).reshape(2, P, OUT)
    )
    gamma = np.asarray(bn_gamma, dtype=np.float32).reshape(2, P).T  # [128, 2]
    beta = np.asarray(bn_beta, dtype=np.float32).reshape(2, P).T
    gb = np.ascontiguousarray(np.concatenate([gamma, beta], axis=1))  # [128, 4]

    nc = _get_nc(with_bias)

    in_maps = []
    for b in range(NCORES):
        m = {
            "x": np.ascontiguousarray(x_bf[b]),
            "wt": wt,
            "wp": wp,
            "wg": wg,
            "gb": gb,
        }
        if with_bias:
            m["bt"] = np.asarray(b_theta, dtype=np.float32).reshape(INTER, 1)
            m["bp"] = np.asarray(b_phi, dtype=np.float32).reshape(INTER, 1)
            m["bg"] = np.asarray(b_g, dtype=np.float32).reshape(1, OUT)
        in_maps.append(m)

    if TRACE:
        _ensure_ntff_hook()
    r = run_bass_kernel_spmd(nc, in_maps, list(range(NCORES)), trace=TRACE)
    LAST_EXEC_NS = r.exec_time_ns

    out = np.stack([r.results[b]["out"] for b in range(NCORES)], axis=0)
    return out.astype(np.float32)


# revision 7
# speedup vs baseline: 1.0104x; 1.0104x over previous
"""Trainium2 Bass kernel for AdaptiveGraphConv (per-(b,t) graph attention + BatchNorm2d).

Reference math (B=8, C=256, T=64, V=468, INTER=128, OUT=256):
    theta = einsum('bctv,ic->btvi', x, W_theta) + b_theta
    phi   = einsum('bctv,ic->btvi', x, W_phi)   + b_phi
    g     = einsum('bctv,oc->btvo', x, W_g)     + b_g
    A     = softmax(theta @ phi^T / sqrt(INTER), axis=-1)   # per (b,t), V x V
    out   = (A @ g) transposed to (B, OUT, T, V)
    out   = batchnorm2d(out, training stats over (B,T,V) per channel)

Sharding: data-parallel over B (1 batch per NeuronCore, 8 cores);
BN batch statistics are all-reduced across cores.

Device program (SPMD, per core, matmuls bf16 with fp32 PSUM accumulation).
Software-pipelined phase 1 as before. Tail restructured vs the original:
  - BN stats are split head (t < T_HEAD) / tail: the head all-reduce is
    launched mid-loop and completes entirely under compute; only the small
    tail all-reduce is exposed at the end.
  - Output is written in bf16 (host upcasts to fp32), halving writeback.
  - Phase 2 works on whole (oc, tb) tiles: one 4x-mode DVE affine + one
    ~1MB DMA per tile.
"""

import math

import numpy as np
import ml_dtypes

import concourse.bacc as bacc
import concourse.tile as tile
from concourse import mybir
from concourse.bass_utils import run_bass_kernel_spmd

B, C, T, V = 8, 256, 64, 468
INTER, OUT = 128, 256
BN_EPS = 1e-5
NCORES = 8
P = 128

SCALE = 1.0 / math.sqrt(INTER)
# w-axis chunks of V for 128-partition tiles
WCH = [(0, 128), (128, 128), (256, 128), (384, V - 384)]
T_BLK = 8  # t-slices per input DMA
N_TB = T // T_BLK
T_HEAD = 48     # head-stats cutoff; head all-reduce launched at COLL_ITER
COLL_ITER = 50  # loop iteration at which head aggregation+collective is emitted

F32 = mybir.dt.float32
BF16 = mybir.dt.bfloat16

TRACE = False
LAST_EXEC_NS = None

_CACHE = {}


def _build(with_bias: bool):
    nc = bacc.Bacc("TRN2", target_bir_lowering=False, debug=False, num_devices=NCORES)

    x_ext = nc.dram_tensor("x", [C, T, V], BF16, kind="ExternalInput").ap()
    wt_ext = nc.dram_tensor("wt", [2, P, INTER], BF16, kind="ExternalInput").ap()
    wp_ext = nc.dram_tensor("wp", [2, P, INTER], BF16, kind="ExternalInput").ap()
    wg_ext = nc.dram_tensor("wg", [2, P, OUT], BF16, kind="ExternalInput").ap()
    gb_ext = nc.dram_tensor("gb", [P, 4], F32, kind="ExternalInput").ap()
    if with_bias:
        bt_ext = nc.dram_tensor("bt", [INTER, 1], F32, kind="ExternalInput").ap()
        bp_ext = nc.dram_tensor("bp", [INTER, 1], F32, kind="ExternalInput").ap()
        bg_ext = nc.dram_tensor("bg", [1, OUT], F32, kind="ExternalInput").ap()
    out_ext = nc.dram_tensor("out", [OUT, T, V], BF16, kind="ExternalOutput").ap()

    cnt_glob = float(NCORES * T * V)
    wsz3 = WCH[3][1]

    with tile.TileContext(nc) as tc:
        with (
            tc.tile_pool(name="consts", bufs=1) as consts,
            tc.tile_pool(name="xin", bufs=2) as xin,
            tc.tile_pool(name="thp", bufs=3) as thp,
            tc.tile_pool(name="ep", bufs=3) as ep,
            tc.tile_pool(name="gp", bufs=3) as gp,
            tc.tile_pool(name="es2", bufs=2) as es2,
            tc.tile_pool(name="es1", bufs=2) as es1,
            tc.tile_pool(name="zp", bufs=2) as zp,
            tc.tile_pool(name="small", bufs=1) as small,
            tc.tile_pool(name="p2", bufs=3) as p2,
            tc.tile_pool(name="pp_a", bufs=1, space="PSUM") as pp_a,
            tc.tile_pool(name="pp_s", bufs=1, space="PSUM") as pp_s,
            tc.tile_pool(name="pp_g", bufs=1, space="PSUM") as pp_g,
            tc.tile_pool(name="pp_u", bufs=2, space="PSUM") as pp_u,
            tc.tile_pool(name="dram", bufs=1, space="DRAM") as dram,
        ):
            # ---- constants ----
            wt_sb = [consts.tile([P, INTER], BF16, tag=f"wt{k}", name=f"wt_sb{k}") for k in range(2)]
            wp_sb = [consts.tile([P, INTER], BF16, tag=f"wp{k}", name=f"wp_sb{k}") for k in range(2)]
            wg_sb = [consts.tile([P, OUT], BF16, tag=f"wg{k}", name=f"wg_sb{k}") for k in range(2)]
            ones = consts.tile([P, P], BF16, tag="ones")
            nc.vector.memset(ones[:], 1.0)
            gb_sb = consts.tile([P, 4], F32, tag="gb")
            nc.sync.dma_start(out=gb_sb[:], in_=gb_ext[:])
            eps_sb = consts.tile([P, 1], F32, tag="eps")
            nc.vector.memset(eps_sb[:], BN_EPS)
            warm = consts.tile([P, 1], F32, tag="warm")
            nc.scalar.activation(warm[:], eps_sb[:], mybir.ActivationFunctionType.Exp)
            if with_bias:
                bt_sb = consts.tile([INTER, 1], F32, tag="bt")
                bp_sb = consts.tile([INTER, 1], F32, tag="bp")
                bg_sb = consts.tile([P, OUT], F32, tag="bg")
                nc.sync.dma_start(out=bt_sb[:], in_=bt_ext[:])
                nc.sync.dma_start(out=bp_sb[:], in_=bp_ext[:])
                nc.sync.dma_start(out=bg_sb[:], in_=bg_ext.to_broadcast([P, OUT]))

            # per-channel running stats (bn_stats 6-tuples per t-slice and o-chunk)
            stats_head = consts.tile([P, T_HEAD, 2, 6], F32, tag="stats_h", name="stats_head")
            stats_tail = consts.tile([P, T - T_HEAD, 2, 6], F32, tag="stats_t", name="stats_tail")

            obig = {
                (oc, tb): consts.tile([P, T_BLK, V], BF16, tag=f"obig{oc}_{tb}",
                                      name=f"obig{oc}_{tb}")
                for oc in range(2) for tb in range(N_TB)
            }

            # ---- phase 1 (software-pipelined) ----
            xgs = {}

            def load_group(tbi, tsplits=None):
                if tbi >= N_TB or tbi in xgs:
                    return
                xg = xin.tile([P, 2, T_BLK, V], BF16, tag="xg", name=f"xg{tbi}")
                for t0, t1 in tsplits or ((0, T_BLK),):
                    for k in range(2):
                        nc.sync.dma_start(
                            out=xg[:, k, t0:t1, :],
                            in_=x_ext[k * P : (k + 1) * P,
                                      tbi * T_BLK + t0 : tbi * T_BLK + t1, :],
                        )
                xgs[tbi] = xg

            def thph_stage(t):
                # thetaT / phiT : [INTER, V] in one 2-bank PSUM tile
                xg = xgs[t // T_BLK]
                xt = [xg[:, k, t % T_BLK, :] for k in range(2)]
                a_ps = pp_a.tile([P, 2, 512], F32, tag="a")
                for k in range(2):
                    nc.tensor.matmul(
                        a_ps[:, 0, :V], lhsT=wt_sb[k][:], rhs=xt[k],
                        start=(k == 0), stop=(k == 1),
                    )
                for k in range(2):
                    nc.tensor.matmul(
                        a_ps[:, 1, :V], lhsT=wp_sb[k][:], rhs=xt[k],
                        start=(k == 0), stop=(k == 1),
                    )
                tp_sb = thp.tile([P, 2, V], BF16, tag="th_sb")
                if with_bias:
                    nc.scalar.activation(
                        tp_sb[:, 0, :], a_ps[:, 0, :V],
                        mybir.ActivationFunctionType.Identity,
                        bias=bt_sb[:, 0:1],
                    )
                    nc.scalar.activation(
                        tp_sb[:, 1, :], a_ps[:, 1, :V],
                        mybir.ActivationFunctionType.Identity,
                        bias=bp_sb[:, 0:1],
                    )
                else:
                    nc.scalar.copy(tp_sb[:], a_ps[:, :, :V])
                return xt, tp_sb

            def z_block(es1t_p, u_ps_p, t_p):
                # Z matmul + normalize + stats for iteration t_p, emitted one
                # iteration later so the esum fold chain never stalls the PE
                z_ps = pp_a.tile([P, 512], F32, tag="a")
                nc.tensor.matmul(
                    z_ps[:, :V], lhsT=ones[:], rhs=es1t_p[:], start=True, stop=True
                )
                zinv = zp.tile([P, V], F32, tag="zinv")
                nc.vector.reciprocal_approx_fast(out=zinv[:], in_=z_ps[:, :V])
                st = stats_head if t_p < T_HEAD else stats_tail
                ti = t_p if t_p < T_HEAD else t_p - T_HEAD
                tb_p, tt_p = t_p // T_BLK, t_p % T_BLK
                for oc in range(2):
                    o_ap = obig[(oc, tb_p)][:, tt_p, :]
                    nc.vector.tensor_tensor(
                        o_ap, u_ps_p[oc][:, :V], zinv[:], mybir.AluOpType.mult
                    )
                    nc.vector.bn_stats(out=st[:, ti, oc, :], in_=o_ap)

            def emit_payload_oc(pay, st_t, tcnt, oc):
                # pay[:, oc] = sum, pay[:, 2+oc] = sum-of-squares for o-chunk oc
                cnt = float(tcnt * V)
                mv = small.tile([P, 2], F32, tag="mv", name=f"mv_{pay.name}{oc}")
                nc.vector.bn_aggr(out=mv[:], in_=st_t[:, :, oc, :])
                q_p = small.tile([P, 1], F32, tag="q", name=f"q_{pay.name}{oc}")
                nc.vector.tensor_scalar_mul(pay[:, oc : oc + 1], mv[:, 0:1], cnt)
                nc.vector.tensor_tensor(
                    q_p[:], mv[:, 0:1], mv[:, 0:1], mybir.AluOpType.mult
                )
                nc.vector.tensor_tensor(
                    q_p[:], mv[:, 1:2], q_p[:], mybir.AluOpType.add
                )
                nc.vector.tensor_scalar_mul(pay[:, 2 + oc : 3 + oc], q_p[:], cnt)

            def emit_payload(pay, st_t, tcnt):
                emit_payload_oc(pay, st_t, tcnt, 0)
                emit_payload_oc(pay, st_t, tcnt, 1)

            def emit_collective(pay):
                pay_dram = dram.tile([P, 4], F32)
                red_dram = dram.tile([P, 4], F32)
                nc.sync.dma_start(out=pay_dram[:], in_=pay[:])
                nc.gpsimd.collective_compute(
                    "AllReduce",
                    mybir.AluOpType.add,
                    replica_groups=[list(range(NCORES))],
                    ins=[pay_dram.opt()],
                    outs=[red_dram.opt()],
                )
                red = small.tile([P, 4], F32, tag=f"red_{pay.name}", name=f"red_{pay.name}")
                nc.sync.dma_start(out=red[:], in_=red_dram[:])
                return red

            # first x wave ahead of the weight DMAs so theta/phi(0) start ASAP
            xg0 = xin.tile([P, 2, T_BLK, V], BF16, tag="xg", name="xg0")
            for k in range(2):
                nc.sync.dma_start(out=xg0[:, k, 0:1, :],
                                  in_=x_ext[k * P : (k + 1) * P, 0:1, :])
            for k in range(2):
                nc.sync.dma_start(out=wt_sb[k][:], in_=wt_ext[k])
                nc.sync.dma_start(out=wp_sb[k][:], in_=wp_ext[k])
                nc.sync.dma_start(out=wg_sb[k][:], in_=wg_ext[k])
            for k in range(2):
                nc.sync.dma_start(out=xg0[:, k, 1:T_BLK, :],
                                  in_=x_ext[k * P : (k + 1) * P, 1:T_BLK, :])
            xgs[0] = xg0
            load_group(1)
            cur = thph_stage(0)
            pend = None  # (es1t, u_ps, t) of the previous iteration
            red_h = None
            pay_h = None

            for t in range(T):
                tb, tt = t // T_BLK, t % T_BLK

                # head stats (t < T_HEAD) all-reduce, hidden under compute;
                # payload built over several iterations to soften the DVE bump
                if t == COLL_ITER:
                    pay_h = small.tile([P, 4], F32, tag="pay_h", name="pay_h")
                    emit_payload_oc(pay_h, stats_head, T_HEAD, 0)
                elif t == COLL_ITER + 1:
                    emit_payload_oc(pay_h, stats_head, T_HEAD, 1)
                elif t == COLL_ITER + 2:
                    red_h = emit_collective(pay_h)

                xt, tp_sb = cur
                th_sb = tp_sb[:, 0, :]
                ph_sb = tp_sb[:, 1, :]

                # scores^T chunk pairs in 2-bank PSUM tiles; exp per pair.
                # rows [wsz3:] of chunk 3 hold exp(stale-PSUM) garbage; every
                # consumer below slices [:wsz3] for chunk 3, so they never read it
                e_t = ep.tile([P, 4, V], BF16, tag="e")
                s01 = pp_s.tile([P, 2, 512], F32, tag="s")
                for wc in (0, 1):
                    w0, wsz = WCH[wc]
                    nc.tensor.matmul(
                        s01[:wsz, wc, :V], lhsT=ph_sb[:, w0 : w0 + wsz], rhs=th_sb,
                        start=True, stop=True,
                    )
                nc.scalar.activation(
                    e_t[:, 0:2, :], s01[:, :, :V],
                    mybir.ActivationFunctionType.Exp, scale=SCALE,
                )

                if pend is not None:
                    z_block(*pend)
                    pend = None

                # g chunks: [w, OUT] quadrants in one 2-bank PSUM tile
                g_ps = pp_g.tile([P, 2, 2, OUT], F32, tag="g")
                for a in range(2):
                    for b_ in range(2):
                        w0, wsz = WCH[2 * a + b_]
                        for k in range(2):
                            nc.tensor.matmul(
                                g_ps[:wsz, a, b_, :],
                                lhsT=xt[k][:, w0 : w0 + wsz], rhs=wg_sb[k][:],
                                start=(k == 0), stop=(k == 1),
                            )
                g_t = gp.tile([P, 2, 2, OUT], BF16, tag="g")
                if with_bias:
                    for a in range(2):
                        for b_ in range(2):
                            wsz = WCH[2 * a + b_][1]
                            nc.vector.tensor_tensor(
                                g_t[:wsz, a, b_, :], g_ps[:wsz, a, b_, :],
                                bg_sb[:wsz], mybir.AluOpType.add,
                            )
                else:
                    nc.scalar.copy(g_t[:], g_ps[:])

                s23 = pp_s.tile([P, 2, 512], F32, tag="s")
                for wc in (2, 3):
                    w0, wsz = WCH[wc]
                    nc.tensor.matmul(
                        s23[:wsz, wc - 2, :V], lhsT=ph_sb[:, w0 : w0 + wsz], rhs=th_sb,
                        start=True, stop=True,
                    )
                nc.scalar.activation(
                    e_t[:, 2:4, :], s23[:, :, :V],
                    mybir.ActivationFunctionType.Exp, scale=SCALE,
                )

                # softmax denominator pre-folds: e0+e1 (GpSimd, right after
                # exp01), +e2 (DVE), then an in-place partial add of chunk 3's
                # valid rows so a single K=128 ones-matmul computes Z
                es2t = es2.tile([P, V], BF16, tag="es2")
                nc.gpsimd.tensor_tensor(
                    es2t[:], e_t[:, 0, :], e_t[:, 1, :], mybir.AluOpType.add
                )
                es1t = es1.tile([P, V], BF16, tag="es1")
                nc.vector.tensor_tensor(
                    es1t[:], es2t[:], e_t[:, 2, :], mybir.AluOpType.add
                )
                nc.gpsimd.tensor_tensor(
                    es1t[:wsz3], es1t[:wsz3], e_t[:wsz3, 3, :],
                    mybir.AluOpType.add,
                )

                # theta/phi for t+1 ride in the shadow of exp/g-copy of t
                if tt == T_BLK - 1:
                    load_group(tb + 2)
                if t + 1 < T:
                    cur = thph_stage(t + 1)

                # U^T = E_unnorm @ g : [OUT(2x128), V]
                u_ps = []
                for oc in range(2):
                    up = pp_u.tile([P, 512], F32, tag="u")
                    for wc in range(4):
                        a, b_ = wc // 2, wc % 2
                        ksz = WCH[wc][1]
                        nc.tensor.matmul(
                            up[:, :V],
                            lhsT=g_t[:ksz, a, b_, oc * P : (oc + 1) * P],
                            rhs=e_t[:ksz, wc, :],
                            start=(wc == 0), stop=(wc == 3),
                        )
                    u_ps.append(up)

                pend = (es1t, u_ps, t)

            # epilogue: normalize + stats for the final t-slice
            z_block(*pend)
            pend = None

            # tail stats all-reduce (small, exposed)
            pay_t = small.tile([P, 4], F32, tag="pay_t", name="pay_t")
            emit_payload(pay_t, stats_tail, T - T_HEAD)
            red_t = emit_collective(pay_t)

            # mean = sum/N ; var = sumsq/N - mean^2 ; s = gamma/sqrt(var+eps)
            # t = beta - mean*s
            red = small.tile([P, 4], F32, tag="red")
            nc.vector.tensor_tensor(red[:], red_h[:], red_t[:], mybir.AluOpType.add)
            mean_g = small.tile([P, 2], F32, tag="mean_g")
            ex2 = small.tile([P, 2], F32, tag="ex2")
            var_g = small.tile([P, 2], F32, tag="var_g")
            rstd = small.tile([P, 2], F32, tag="rstd")
            s_vec = small.tile([P, 2], F32, tag="s_vec")
            t_vec = small.tile([P, 2], F32, tag="t_vec")
            nc.vector.tensor_scalar_mul(mean_g[:], red[:, 0:2], 1.0 / cnt_glob)
            nc.vector.tensor_scalar_mul(ex2[:], red[:, 2:4], 1.0 / cnt_glob)
            nc.vector.tensor_tensor(
                var_g[:], mean_g[:], mean_g[:], mybir.AluOpType.mult
            )
            nc.vector.tensor_tensor(
                var_g[:], ex2[:], var_g[:], mybir.AluOpType.subtract
            )
            nc.scalar.activation(
                rstd[:], var_g[:], mybir.ActivationFunctionType.Sqrt,
                bias=eps_sb[:, 0:1],
            )
            nc.vector.reciprocal(out=rstd[:], in_=rstd[:])
            nc.vector.tensor_tensor(s_vec[:], rstd[:], gb_sb[:, 0:2], mybir.AluOpType.mult)
            nc.vector.tensor_tensor(t_vec[:], mean_g[:], s_vec[:], mybir.AluOpType.mult)
            nc.vector.tensor_tensor(
                t_vec[:], gb_sb[:, 2:4], t_vec[:], mybir.AluOpType.subtract
            )

            # ---- phase 2: whole-tile affine (DVE 4x mode) + bf16 writeback ----
            for tb in range(N_TB):
                for oc in range(2):
                    tin = obig[(oc, tb)]
                    tout = p2.tile([P, T_BLK, V], BF16, tag="p2out",
                                   name=f"tout{oc}_{tb}")
                    nc.vector.tensor_scalar(
                        tout[:], tin[:],
                        s_vec[:, oc : oc + 1], t_vec[:, oc : oc + 1],
                        mybir.AluOpType.mult, mybir.AluOpType.add,
                    )
                    nc.sync.dma_start(
                        out=out_ext[oc * P : (oc + 1) * P,
                                    tb * T_BLK : (tb + 1) * T_BLK, :],
                        in_=tout[:],
                    )

    nc.compile()
    return nc


def _get_nc(with_bias: bool):
    key = with_bias
    if key not in _CACHE:
        _CACHE[key] = _build(with_bias)
    return _CACHE[key]


def _ensure_ntff_hook():
    import sys, types
    import antenv

    if "antenv.axon_hooks" not in sys.modules:
        mod = types.ModuleType("antenv.axon_hooks")
        _h = [None]
        mod.set_axon_ntff_profile_hook = lambda h: _h.__setitem__(0, h)
        mod.get_axon_ntff_profile_hook = lambda: _h[0]
        sys.modules["antenv.axon_hooks"] = mod
        antenv.axon_hooks = mod
    mod = sys.modules["antenv.axon_hooks"]
    if mod.get_axon_ntff_profile_hook() is None:
        try:
            from trn_agent_boot.trn_boot import _ntff_profile_via_ctypes

            mod.set_axon_ntff_profile_hook(
                _ntff_profile_via_ctypes("/opt/axon/libaxon_pjrt.so")
            )
        except Exception:
            pass


def kernel(x, W_theta, b_theta, W_phi, b_phi, W_g, b_g, bn_gamma, bn_beta):
    global LAST_EXEC_NS
    x = np.asarray(x, dtype=np.float32)
    with_bias = bool(
        np.any(np.asarray(b_theta)) or np.any(np.asarray(b_phi)) or np.any(np.asarray(b_g))
    )

    x_bf = x.astype(ml_dtypes.bfloat16)  # (B, C, T, V)
    wt = np.ascontiguousarray(
        np.asarray(W_theta, dtype=np.float32).T.astype(ml_dtypes.bfloat16).reshape(2, P, INTER)
    )
    wp = np.ascontiguousarray(
        np.asarray(W_phi, dtype=np.float32).T.astype(ml_dtypes.bfloat16).reshape(2, P, INTER)
    )
    wg = np.ascontiguousarray(
        np.asarray(W_g, dtype=np.float32).T.astype(ml_dtypes.bfloat16).reshape(2, P, OUT)
    )
    gamma = np.asarray(bn_gamma, dtype=np.float32).reshape(2, P).T  # [128, 2]
    beta = np.asarray(bn_beta, dtype=np.float32).reshape(2, P).T
    gb = np.ascontiguousarray(np.concatenate([gamma, beta], axis=1))  # [128, 4]

    nc = _get_nc(with_bias)

    in_maps = []
    for b in range(NCORES):
        m = {
            "x": np.ascontiguousarray(x_bf[b]),
            "wt": wt,
            "wp": wp,
            "wg": wg,
            "gb": gb,
        }
        if with_bias:
            m["bt"] = np.asarray(b_theta, dtype=np.float32).reshape(INTER, 1)
            m["bp"] = np.asarray(b_phi, dtype=np.float32).reshape(INTER, 1)
            m["bg"] = np.asarray(b_g, dtype=np.float32).reshape(1, OUT)
        in_maps.append(m)

    if TRACE:
        _ensure_ntff_hook()
    r = run_bass_kernel_spmd(nc, in_maps, list(range(NCORES)), trace=TRACE)
    LAST_EXEC_NS = r.exec_time_ns

    out = np.stack([r.results[b]["out"] for b in range(NCORES)], axis=0)
    return out.astype(np.float32)


# revision 17
# speedup vs baseline: 1.0240x; 1.0134x over previous
"""Trainium2 Bass kernel for AdaptiveGraphConv (per-(b,t) graph attention + BatchNorm2d).

Reference math (B=8, C=256, T=64, V=468, INTER=128, OUT=256):
    theta = einsum('bctv,ic->btvi', x, W_theta) + b_theta
    phi   = einsum('bctv,ic->btvi', x, W_phi)   + b_phi
    g     = einsum('bctv,oc->btvo', x, W_g)     + b_g
    A     = softmax(theta @ phi^T / sqrt(INTER), axis=-1)   # per (b,t), V x V
    out   = (A @ g) transposed to (B, OUT, T, V)
    out   = batchnorm2d(out, training stats over (B,T,V) per channel)

Sharding: data-parallel over B (1 batch per NeuronCore, 8 cores);
BN batch statistics are all-reduced across cores.

Device program (SPMD, per core, matmuls bf16 with fp32 PSUM accumulation).
Software-pipelined phase 1 as before. Tail restructured vs the original:
  - BN stats are split head (t < T_HEAD) / tail: the head all-reduce is
    launched mid-loop and completes entirely under compute; only the small
    tail all-reduce is exposed at the end.
  - Output is written in bf16 (host upcasts to fp32), halving writeback.
  - Phase 2 works on whole (oc, tb) tiles: one 4x-mode DVE affine + one
    ~1MB DMA per tile.
"""

import math

import numpy as np
import ml_dtypes

import concourse.bacc as bacc
import concourse.tile as tile
from concourse import mybir
from concourse.bass_utils import run_bass_kernel_spmd

B, C, T, V = 8, 256, 64, 468
INTER, OUT = 128, 256
BN_EPS = 1e-5
NCORES = 8
P = 128

SCALE = 1.0 / math.sqrt(INTER)
# w-axis chunks of V for 128-partition tiles
WCH = [(0, 128), (128, 128), (256, 128), (384, V - 384)]
T_BLK = 8  # t-slices per input DMA
N_TB = T // T_BLK
T_HEAD = 48     # head-stats cutoff; head all-reduce launched at COLL_ITER
COLL_ITER = 50  # loop iteration at which head aggregation+collective is emitted

# fp8 DoubleRow theta/phi: weights pre-scaled by 64 and x by 4 (keeps e4m3
# out of its subnormal range); the 2^16 product scale is folded exactly into
# the exp's scale factor.
FP8_THPH = True
XS_SCALE = 4.0
WS_SCALE = 64.0
PROD_SCALE = float(XS_SCALE * WS_SCALE) ** 2  # 65536

F32 = mybir.dt.float32
BF16 = mybir.dt.bfloat16
F8E4 = mybir.dt.float8e4

TRACE = False
LAST_EXEC_NS = None

_CACHE = {}


def _build(with_bias: bool):
    nc = bacc.Bacc("TRN2", target_bir_lowering=False, debug=False, num_devices=NCORES)

    x_ext = nc.dram_tensor("x", [C, T, V], BF16, kind="ExternalInput").ap()
    wg_ext = nc.dram_tensor("wg", [2, P, OUT], BF16, kind="ExternalInput").ap()
    use_fp8 = FP8_THPH and not with_bias
    if use_fp8:
        x8_ext = nc.dram_tensor("x8", [C, T, V], F8E4, kind="ExternalInput").ap()
        wt_ext = nc.dram_tensor("wt", [P, 2, INTER], F8E4, kind="ExternalInput").ap()
        wp_ext = nc.dram_tensor("wp", [P, 2, INTER], F8E4, kind="ExternalInput").ap()
    else:
        wt_ext = nc.dram_tensor("wt", [2, P, INTER], BF16, kind="ExternalInput").ap()
        wp_ext = nc.dram_tensor("wp", [2, P, INTER], BF16, kind="ExternalInput").ap()
    gb_ext = nc.dram_tensor("gb", [P, 4], F32, kind="ExternalInput").ap()
    if with_bias:
        bt_ext = nc.dram_tensor("bt", [INTER, 1], F32, kind="ExternalInput").ap()
        bp_ext = nc.dram_tensor("bp", [INTER, 1], F32, kind="ExternalInput").ap()
        bg_ext = nc.dram_tensor("bg", [1, OUT], F32, kind="ExternalInput").ap()
    out_ext = nc.dram_tensor("out", [OUT, T, V], BF16, kind="ExternalOutput").ap()

    cnt_glob = float(NCORES * T * V)
    wsz3 = WCH[3][1]
    escale = SCALE / PROD_SCALE if use_fp8 else SCALE

    with tile.TileContext(nc) as tc:
        with (
            tc.tile_pool(name="consts", bufs=1) as consts,
            tc.tile_pool(name="xin", bufs=2) as xin,
            tc.tile_pool(name="xin8", bufs=2) as xin8,
            tc.tile_pool(name="thp", bufs=3) as thp,
            tc.tile_pool(name="ep", bufs=2) as ep,
            tc.tile_pool(name="gp", bufs=3) as gp,
            tc.tile_pool(name="es2", bufs=2) as es2,
            tc.tile_pool(name="es1", bufs=2) as es1,
            tc.tile_pool(name="zp", bufs=2) as zp,
            tc.tile_pool(name="small", bufs=1) as small,
            tc.tile_pool(name="p2", bufs=2) as p2,
            tc.tile_pool(name="pp_a", bufs=1, space="PSUM") as pp_a,
            tc.tile_pool(name="pp_s", bufs=1, space="PSUM") as pp_s,
            tc.tile_pool(name="pp_g", bufs=1, space="PSUM") as pp_g,
            tc.tile_pool(name="pp_u", bufs=2, space="PSUM") as pp_u,
            tc.tile_pool(name="dram", bufs=1, space="DRAM") as dram,
        ):
            # ---- constants ----
            if use_fp8:
                wt8_sb = consts.tile([P, 2, INTER], F8E4, tag="wt8", name="wt8_sb")
                wp8_sb = consts.tile([P, 2, INTER], F8E4, tag="wp8", name="wp8_sb")
            else:
                wt_sb = [consts.tile([P, INTER], BF16, tag=f"wt{k}", name=f"wt_sb{k}") for k in range(2)]
                wp_sb = [consts.tile([P, INTER], BF16, tag=f"wp{k}", name=f"wp_sb{k}") for k in range(2)]
            wg_sb = [consts.tile([P, OUT], BF16, tag=f"wg{k}", name=f"wg_sb{k}") for k in range(2)]
            ones = consts.tile([P, P], BF16, tag="ones")
            nc.vector.memset(ones[:], 1.0)
            gb_sb = consts.tile([P, 4], F32, tag="gb")
            nc.sync.dma_start(out=gb_sb[:], in_=gb_ext[:])
            eps_sb = consts.tile([P, 1], F32, tag="eps")
            nc.vector.memset(eps_sb[:], BN_EPS)
            warm = consts.tile([P, 1], F32, tag="warm")
            nc.scalar.activation(warm[:], eps_sb[:], mybir.ActivationFunctionType.Exp)
            if with_bias:
                bt_sb = consts.tile([INTER, 1], F32, tag="bt")
                bp_sb = consts.tile([INTER, 1], F32, tag="bp")
                bg_sb = consts.tile([P, OUT], F32, tag="bg")
                nc.sync.dma_start(out=bt_sb[:], in_=bt_ext[:])
                nc.sync.dma_start(out=bp_sb[:], in_=bp_ext[:])
                nc.sync.dma_start(out=bg_sb[:], in_=bg_ext.to_broadcast([P, OUT]))

            # per-channel running stats (bn_stats 6-tuples per t-slice and o-chunk)
            stats_head = consts.tile([P, T_HEAD, 2, 6], F32, tag="stats_h", name="stats_head")
            stats_tail = consts.tile([P, T - T_HEAD, 2, 6], F32, tag="stats_t", name="stats_tail")

            obig = {
                (oc, tb): consts.tile([P, T_BLK, V], BF16, tag=f"obig{oc}_{tb}",
                                      name=f"obig{oc}_{tb}")
                for oc in range(2) for tb in range(N_TB)
            }

            # ---- phase 1 (software-pipelined) ----
            xgs = {}

            def load_group(tbi, tsplits=None):
                if tbi >= N_TB or tbi in xgs:
                    return
                xg = xin.tile([P, 2, T_BLK, V], BF16, tag="xg", name=f"xg{tbi}")
                xg8 = None
                if use_fp8:
                    xg8 = xin8.tile([P, 2, T_BLK, V], F8E4, tag="xg8",
                                    name=f"xg8_{tbi}")
                for t0, t1 in tsplits or ((0, T_BLK),):
                    for k in range(2):
                        if use_fp8:
                            nc.sync.dma_start(
                                out=xg8[:, k, t0:t1, :],
                                in_=x8_ext[k * P : (k + 1) * P,
                                           tbi * T_BLK + t0 : tbi * T_BLK + t1, :],
                            )
                        nc.sync.dma_start(
                            out=xg[:, k, t0:t1, :],
                            in_=x_ext[k * P : (k + 1) * P,
                                      tbi * T_BLK + t0 : tbi * T_BLK + t1, :],
                        )
                xgs[tbi] = (xg, xg8)

            def thph_stage(t):
                # thetaT / phiT : [INTER, V] in one 2-bank PSUM tile
                xg, xg8 = xgs[t // T_BLK]
                xt = [xg[:, k, t % T_BLK, :] for k in range(2)]
                a_ps = pp_a.tile([P, 2, 512], F32, tag="a")
                if use_fp8:
                    # fp8 DoubleRow: both 128-row k-chunks in one instruction
                    x8t = xg8[:, :, t % T_BLK, :]
                    nc.tensor.matmul(
                        a_ps[:, 0, :V], lhsT=wt8_sb[:], rhs=x8t,
                        start=True, stop=True,
                        perf_mode=mybir.MatmulPerfMode.DoubleRow,
                    )
                    nc.tensor.matmul(
                        a_ps[:, 1, :V], lhsT=wp8_sb[:], rhs=x8t,
                        start=True, stop=True,
                        perf_mode=mybir.MatmulPerfMode.DoubleRow,
                    )
                else:
                    for k in range(2):
                        nc.tensor.matmul(
                            a_ps[:, 0, :V], lhsT=wt_sb[k][:], rhs=xt[k],
                            start=(k == 0), stop=(k == 1),
                        )
                    for k in range(2):
                        nc.tensor.matmul(
                            a_ps[:, 1, :V], lhsT=wp_sb[k][:], rhs=xt[k],
                            start=(k == 0), stop=(k == 1),
                        )
                tp_sb = thp.tile([P, 2, V], BF16, tag="th_sb")
                if with_bias:
                    nc.scalar.activation(
                        tp_sb[:, 0, :], a_ps[:, 0, :V],
                        mybir.ActivationFunctionType.Identity,
                        bias=bt_sb[:, 0:1],
                    )
                    nc.scalar.activation(
                        tp_sb[:, 1, :], a_ps[:, 1, :V],
                        mybir.ActivationFunctionType.Identity,
                        bias=bp_sb[:, 0:1],
                    )
                else:
                    nc.scalar.copy(tp_sb[:], a_ps[:, :, :V])
                return xt, tp_sb

            def z_block(es1t_p, u_ps_p, t_p):
                # Z matmul + normalize + stats for iteration t_p, emitted one
                # iteration later so the esum fold chain never stalls the PE
                z_ps = pp_a.tile([P, 512], F32, tag="a")
                nc.tensor.matmul(
                    z_ps[:, :V], lhsT=ones[:], rhs=es1t_p[:], start=True, stop=True
                )
                zinv = zp.tile([P, V], F32, tag="zinv")
                nc.vector.reciprocal_approx_fast(out=zinv[:], in_=z_ps[:, :V])
                st = stats_head if t_p < T_HEAD else stats_tail
                ti = t_p if t_p < T_HEAD else t_p - T_HEAD
                tb_p, tt_p = t_p // T_BLK, t_p % T_BLK
                for oc in range(2):
                    o_ap = obig[(oc, tb_p)][:, tt_p, :]
                    nc.vector.tensor_tensor(
                        o_ap, u_ps_p[oc][:, :V], zinv[:], mybir.AluOpType.mult
                    )
                    nc.vector.bn_stats(out=st[:, ti, oc, :], in_=o_ap)

            def emit_payload_oc(pay, st_t, tcnt, oc):
                # pay[:, oc] = sum, pay[:, 2+oc] = sum-of-squares for o-chunk oc
                cnt = float(tcnt * V)
                mv = small.tile([P, 2], F32, tag="mv", name=f"mv_{pay.name}{oc}")
                nc.vector.bn_aggr(out=mv[:], in_=st_t[:, :, oc, :])
                q_p = small.tile([P, 1], F32, tag="q", name=f"q_{pay.name}{oc}")
                nc.vector.tensor_scalar_mul(pay[:, oc : oc + 1], mv[:, 0:1], cnt)
                nc.vector.tensor_tensor(
                    q_p[:], mv[:, 0:1], mv[:, 0:1], mybir.AluOpType.mult
                )
                nc.vector.tensor_tensor(
                    q_p[:], mv[:, 1:2], q_p[:], mybir.AluOpType.add
                )
                nc.vector.tensor_scalar_mul(pay[:, 2 + oc : 3 + oc], q_p[:], cnt)

            def emit_payload(pay, st_t, tcnt):
                emit_payload_oc(pay, st_t, tcnt, 0)
                emit_payload_oc(pay, st_t, tcnt, 1)

            def emit_collective(pay):
                pay_dram = dram.tile([P, 4], F32)
                red_dram = dram.tile([P, 4], F32)
                nc.sync.dma_start(out=pay_dram[:], in_=pay[:])
                nc.gpsimd.collective_compute(
                    "AllReduce",
                    mybir.AluOpType.add,
                    replica_groups=[list(range(NCORES))],
                    ins=[pay_dram.opt()],
                    outs=[red_dram.opt()],
                )
                red = small.tile([P, 4], F32, tag=f"red_{pay.name}", name=f"red_{pay.name}")
                nc.sync.dma_start(out=red[:], in_=red_dram[:])
                return red

            # first x wave ahead of the weight DMAs so theta/phi(0) start ASAP
            xg0 = xin.tile([P, 2, T_BLK, V], BF16, tag="xg", name="xg0")
            xg80 = None
            if use_fp8:
                xg80 = xin8.tile([P, 2, T_BLK, V], F8E4, tag="xg8", name="xg8_0")
                for k in range(2):
                    nc.sync.dma_start(out=xg80[:, k, 0:1, :],
                                      in_=x8_ext[k * P : (k + 1) * P, 0:1, :])
                nc.sync.dma_start(out=wt8_sb[:], in_=wt_ext[:])
                nc.sync.dma_start(out=wp8_sb[:], in_=wp_ext[:])
            for k in range(2):
                nc.sync.dma_start(out=xg0[:, k, 0:1, :],
                                  in_=x_ext[k * P : (k + 1) * P, 0:1, :])
            if not use_fp8:
                for k in range(2):
                    nc.sync.dma_start(out=wt_sb[k][:], in_=wt_ext[k])
                    nc.sync.dma_start(out=wp_sb[k][:], in_=wp_ext[k])
            for k in range(2):
                nc.sync.dma_start(out=wg_sb[k][:], in_=wg_ext[k])
            for k in range(2):
                if use_fp8:
                    nc.sync.dma_start(out=xg80[:, k, 1:T_BLK, :],
                                      in_=x8_ext[k * P : (k + 1) * P, 1:T_BLK, :])
                nc.sync.dma_start(out=xg0[:, k, 1:T_BLK, :],
                                  in_=x_ext[k * P : (k + 1) * P, 1:T_BLK, :])
            xgs[0] = (xg0, xg80)
            load_group(1)
            cur = thph_stage(0)
            pend = None  # (es1t, u_ps, t) of the previous iteration
            red_h = None
            pay_h = None

            for t in range(T):
                tb, tt = t // T_BLK, t % T_BLK

                # head stats (t < T_HEAD) all-reduce, hidden under compute;
                # payload built over several iterations to soften the DVE bump
                if t == COLL_ITER:
                    pay_h = small.tile([P, 4], F32, tag="pay_h", name="pay_h")
                    emit_payload_oc(pay_h, stats_head, T_HEAD, 0)
                elif t == COLL_ITER + 1:
                    emit_payload_oc(pay_h, stats_head, T_HEAD, 1)
                elif t == COLL_ITER + 2:
                    red_h = emit_collective(pay_h)

                xt, tp_sb = cur
                th_sb = tp_sb[:, 0, :]
                ph_sb = tp_sb[:, 1, :]

                # scores^T chunk pairs in 2-bank PSUM tiles; exp per pair.
                # rows [wsz3:] of chunk 3 hold exp(stale-PSUM) garbage; every
                # consumer below slices [:wsz3] for chunk 3, so they never read it
                e_t = ep.tile([P, 4, V], BF16, tag="e")
                s01 = pp_s.tile([P, 2, 512], F32, tag="s")
                for wc in (0, 1):
                    w0, wsz = WCH[wc]
                    nc.tensor.matmul(
                        s01[:wsz, wc, :V], lhsT=ph_sb[:, w0 : w0 + wsz], rhs=th_sb,
                        start=True, stop=True,
                    )
                nc.scalar.activation(
                    e_t[:, 0:2, :], s01[:, :, :V],
                    mybir.ActivationFunctionType.Exp, scale=escale,
                )

                if pend is not None:
                    z_block(*pend)
                    pend = None

                # g chunks: [w, OUT] quadrants in one 2-bank PSUM tile
                g_ps = pp_g.tile([P, 2, 2, OUT], F32, tag="g")
                for a in range(2):
                    for b_ in range(2):
                        w0, wsz = WCH[2 * a + b_]
                        for k in range(2):
                            nc.tensor.matmul(
                                g_ps[:wsz, a, b_, :],
                                lhsT=xt[k][:, w0 : w0 + wsz], rhs=wg_sb[k][:],
                                start=(k == 0), stop=(k == 1),
                            )
                g_t = gp.tile([P, 2, 2, OUT], BF16, tag="g")
                if with_bias:
                    for a in range(2):
                        for b_ in range(2):
                            wsz = WCH[2 * a + b_][1]
                            nc.vector.tensor_tensor(
                                g_t[:wsz, a, b_, :], g_ps[:wsz, a, b_, :],
                                bg_sb[:wsz], mybir.AluOpType.add,
                            )
                else:
                    nc.scalar.copy(g_t[:], g_ps[:])

                s23 = pp_s.tile([P, 2, 512], F32, tag="s")
                for wc in (2, 3):
                    w0, wsz = WCH[wc]
                    nc.tensor.matmul(
                        s23[:wsz, wc - 2, :V], lhsT=ph_sb[:, w0 : w0 + wsz], rhs=th_sb,
                        start=True, stop=True,
                    )
                nc.scalar.activation(
                    e_t[:, 2:4, :], s23[:, :, :V],
                    mybir.ActivationFunctionType.Exp, scale=escale,
                )

                # softmax denominator pre-folds: e0+e1 (GpSimd, right after
                # exp01), +e2 (DVE), then an in-place partial add of chunk 3's
                # valid rows so a single K=128 ones-matmul computes Z
                es2t = es2.tile([P, V], BF16, tag="es2")
                nc.gpsimd.tensor_tensor(
                    es2t[:], e_t[:, 0, :], e_t[:, 1, :], mybir.AluOpType.add
                )
                es1t = es1.tile([P, V], BF16, tag="es1")
                nc.vector.tensor_tensor(
                    es1t[:], es2t[:], e_t[:, 2, :], mybir.AluOpType.add
                )
                nc.gpsimd.tensor_tensor(
                    es1t[:wsz3], es1t[:wsz3], e_t[:wsz3, 3, :],
                    mybir.AluOpType.add,
                )

                # theta/phi for t+1 ride in the shadow of exp/g-copy of t
                if tt == T_BLK - 1:
                    load_group(tb + 2)
                if t + 1 < T:
                    cur = thph_stage(t + 1)

                # U^T = E_unnorm @ g : [OUT(2x128), V]
                u_ps = []
                for oc in range(2):
                    up = pp_u.tile([P, 512], F32, tag="u")
                    for wc in range(4):
                        a, b_ = wc // 2, wc % 2
                        ksz = WCH[wc][1]
                        nc.tensor.matmul(
                            up[:, :V],
                            lhsT=g_t[:ksz, a, b_, oc * P : (oc + 1) * P],
                            rhs=e_t[:ksz, wc, :],
                            start=(wc == 0), stop=(wc == 3),
                        )
                    u_ps.append(up)

                pend = (es1t, u_ps, t)

            # epilogue: normalize + stats for the final t-slice
            z_block(*pend)
            pend = None

            # tail stats all-reduce (small, exposed)
            pay_t = small.tile([P, 4], F32, tag="pay_t", name="pay_t")
            emit_payload(pay_t, stats_tail, T - T_HEAD)
            red_t = emit_collective(pay_t)

            # mean = sum/N ; var = sumsq/N - mean^2 ; s = gamma/sqrt(var+eps)
            # t = beta - mean*s
            red = small.tile([P, 4], F32, tag="red")
            nc.vector.tensor_tensor(red[:], red_h[:], red_t[:], mybir.AluOpType.add)
            mean_g = small.tile([P, 2], F32, tag="mean_g")
            ex2 = small.tile([P, 2], F32, tag="ex2")
            var_g = small.tile([P, 2], F32, tag="var_g")
            rstd = small.tile([P, 2], F32, tag="rstd")
            s_vec = small.tile([P, 2], F32, tag="s_vec")
            t_vec = small.tile([P, 2], F32, tag="t_vec")
            nc.vector.tensor_scalar_mul(mean_g[:], red[:, 0:2], 1.0 / cnt_glob)
            nc.vector.tensor_scalar_mul(ex2[:], red[:, 2:4], 1.0 / cnt_glob)
            nc.vector.tensor_tensor(
                var_g[:], mean_g[:], mean_g[:], mybir.AluOpType.mult
            )
            nc.vector.tensor_tensor(
                var_g[:], ex2[:], var_g[:], mybir.AluOpType.subtract
            )
            nc.scalar.activation(
                rstd[:], var_g[:], mybir.ActivationFunctionType.Sqrt,
                bias=eps_sb[:, 0:1],
            )
            nc.vector.reciprocal(out=rstd[:], in_=rstd[:])
            nc.vector.tensor_tensor(s_vec[:], rstd[:], gb_sb[:, 0:2], mybir.AluOpType.mult)
            nc.vector.tensor_tensor(t_vec[:], mean_g[:], s_vec[:], mybir.AluOpType.mult)
            nc.vector.tensor_tensor(
                t_vec[:], gb_sb[:, 2:4], t_vec[:], mybir.AluOpType.subtract
            )

            # ---- phase 2: whole-tile affine (DVE 4x mode) + bf16 writeback ----
            for tb in range(N_TB):
                for oc in range(2):
                    tin = obig[(oc, tb)]
                    tout = p2.tile([P, T_BLK, V], BF16, tag="p2out",
                                   name=f"tout{oc}_{tb}")
                    nc.vector.tensor_scalar(
                        tout[:], tin[:],
                        s_vec[:, oc : oc + 1], t_vec[:, oc : oc + 1],
                        mybir.AluOpType.mult, mybir.AluOpType.add,
                    )
                    nc.sync.dma_start(
                        out=out_ext[oc * P : (oc + 1) * P,
                                    tb * T_BLK : (tb + 1) * T_BLK, :],
                        in_=tout[:],
                    )

    nc.compile()
    return nc


def _get_nc(with_bias: bool):
    key = with_bias
    if key not in _CACHE:
        _CACHE[key] = _build(with_bias)
    return _CACHE[key]


def _ensure_ntff_hook():
    import sys, types
    import antenv

    if "antenv.axon_hooks" not in sys.modules:
        mod = types.ModuleType("antenv.axon_hooks")
        _h = [None]
        mod.set_axon_ntff_profile_hook = lambda h: _h.__setitem__(0, h)
        mod.get_axon_ntff_profile_hook = lambda: _h[0]
        sys.modules["antenv.axon_hooks"] = mod
        antenv.axon_hooks = mod
    mod = sys.modules["antenv.axon_hooks"]
    if mod.get_axon_ntff_profile_hook() is None:
        try:
            from trn_agent_boot.trn_boot import _ntff_profile_via_ctypes

            mod.set_axon_ntff_profile_hook(
                _ntff_profile_via_ctypes("/opt/axon/libaxon_pjrt.so")
            )
        except Exception:
            pass


def kernel(x, W_theta, b_theta, W_phi, b_phi, W_g, b_g, bn_gamma, bn_beta):
    global LAST_EXEC_NS
    x = np.asarray(x, dtype=np.float32)
    with_bias = bool(
        np.any(np.asarray(b_theta)) or np.any(np.asarray(b_phi)) or np.any(np.asarray(b_g))
    )

    use_fp8 = FP8_THPH and not with_bias
    x_bf = x.astype(ml_dtypes.bfloat16)  # (B, C, T, V)
    if use_fp8:
        x8 = (x * XS_SCALE).astype(ml_dtypes.float8_e4m3)
        wt = np.ascontiguousarray(
            (np.asarray(W_theta, dtype=np.float32).T * WS_SCALE)
            .reshape(2, P, INTER).transpose(1, 0, 2).astype(ml_dtypes.float8_e4m3)
        )
        wp = np.ascontiguousarray(
            (np.asarray(W_phi, dtype=np.float32).T * WS_SCALE)
            .reshape(2, P, INTER).transpose(1, 0, 2).astype(ml_dtypes.float8_e4m3)
        )
    else:
        wt = np.ascontiguousarray(
            np.asarray(W_theta, dtype=np.float32).T.astype(ml_dtypes.bfloat16).reshape(2, P, INTER)
        )
        wp = np.ascontiguousarray(
            np.asarray(W_phi, dtype=np.float32).T.astype(ml_dtypes.bfloat16).reshape(2, P, INTER)
        )
    wg = np.ascontiguousarray(
        np.asarray(W_g, dtype=np.float32).T.astype(ml_dtypes.bfloat16).reshape(2, P, OUT)
    )
    gamma = np.asarray(bn_gamma, dtype=np.float32).reshape(2, P).T  # [128, 2]
    beta = np.asarray(bn_beta, dtype=np.float32).reshape(2, P).T
    gb = np.ascontiguousarray(np.concatenate([gamma, beta], axis=1))  # [128, 4]

    nc = _get_nc(with_bias)

    in_maps = []
    for b in range(NCORES):
        m = {
            "x": np.ascontiguousarray(x_bf[b]),
            "wt": wt,
            "wp": wp,
            "wg": wg,
            "gb": gb,
        }
        if use_fp8:
            m["x8"] = np.ascontiguousarray(x8[b])
        if with_bias:
            m["bt"] = np.asarray(b_theta, dtype=np.float32).reshape(INTER, 1)
            m["bp"] = np.asarray(b_phi, dtype=np.float32).reshape(INTER, 1)
            m["bg"] = np.asarray(b_g, dtype=np.float32).reshape(1, OUT)
        in_maps.append(m)

    if TRACE:
        _ensure_ntff_hook()
    r = run_bass_kernel_spmd(nc, in_maps, list(range(NCORES)), trace=TRACE)
    LAST_EXEC_NS = r.exec_time_ns

    out = np.stack([r.results[b]["out"] for b in range(NCORES)], axis=0)
    return out.astype(np.float32)
